# revision 33
# baseline (speedup 1.0000x reference)
"""Trainium2 Bass kernel for the sparse segment-softmax attention module.

Math: out[k] = segment_softmax((q1[b,i] + q2[b,j]) . v) over segments (b, i).
q1/b-bias terms cancel (softmax shift invariance), so
    out[k] = E[b, j_k] / sum_seg E,   E[b, n] = exp(t2[b, n, :] . g),
    g = W2^T v.  t1/W1/b1 are unused.

Device kernel per NeuronCore (2 of 16 batches, data-parallel over 8 cores):
  - t2 shard streams in bf16 TRANSPOSED (XBAR dma_start_transpose) so the
    PE computes u2 = t2 . g as 8 accumulating [128f x 512n] matmuls per
    batch into psum [1, 512]; 4 PE transposes turn the row into the
    [128, 4] table layout; exp on ACT -> bf16.
  - static Benes-network gather: the per-batch 16384-slot gather by idx_j
    is routed as a 15-bit Benes network (host-routed masks, uint8, one big
    DMA). Both batches stack in the free dim; each stage is TWO in-place
    DVE copy_predicated ops using reversed-stride pair views (bf16 data).
    Stages on addr bits 7..14 are free-dim selects; bits 0..6 run between
    two PE corner-turn transposes. Down stages on bits 7..12 pair
    identical values and are skipped. The bit-14 up stage compacts to the
    sink half; the last 7 stages run half width.
  - windowed softmax normalize (4 segments x 32 per partition) + store.

Output is produced directly in natural nnz order. Host does index routing
(cached by idx_j hash); no GPSIMD custom ops.
"""

import hashlib
import os
from contextlib import ExitStack

import numpy as np

B = 16
N1 = 512
N2 = 512
F2 = 1024
DEG = 32
NNZ = B * N1 * DEG
NCORES = 8
BPC = B // NCORES

# ---------------- Benes network topology (static) ----------------
NET_L = 15
NET_N = 1 << NET_L
NSINK = 16384
D_BITS = [7, 8, 9, 10, 11, 12, 13, 14, 0, 1, 2, 3, 4, 5]
M_BIT = 6
ALL_BITS = D_BITS + [M_BIT] + D_BITS[::-1]  # 29 stages
SKIP_STAGES = set(range(6))                 # identical-value pairs: no device op
N_STAGE = len(ALL_BITS)
HALF_D = (N_STAGE - 1) // 2

# Executed stages, device order:
#   idx 0..1   G0 full, f-bits [6, 7]            (flat bits 13, 14 down)
#   corner turn
#   idx 2..14  G1 full, f-bits [0,1,2,3,4,5,6,5,4,3,2,1,0]
#   corner turn
#   idx 15     G0 compacting up stage (flat bit 14): [128,512] -> [128,256]
#   idx 16..22 G0 half, f-bits [6,5,4,3,2,1,0]    (flat bits 13..7 up)
FULL_FBITS = [6, 7] + [0, 1, 2, 3, 4, 5, 6, 5, 4, 3, 2, 1, 0]  # 15 stages, w=512
HALF_FBITS = [6, 5, 4, 3, 2, 1, 0]                             # 7 stages, w=256
CT_AFTER = {1, 14}
_OFFS = [512 * i for i in range(15)]      # full-stage cross masks
_OFF_CPT_C = 15 * 512                     # compact stage cross mask [128, 256]
_OFF_CPT_S = _OFF_CPT_C + 256             # compact stage straight mask
_OFFS_H = [_OFF_CPT_S + 256 + 256 * i for i in range(7)]
MB_TOT = _OFFS_H[-1] + 256                # 9984

_CACHE: dict = {}


# ---------------- host-side Benes routing ----------------

def _route_benes(cur0, dst0):
    masks = [np.zeros(NET_N, np.uint8) for _ in range(N_STAGE)]
    cur = cur0.astype(np.int64).copy()
    dst = dst0.astype(np.int64).copy()
    items = np.arange(NET_N)
    for depth in range(HALF_D):
        t = ALL_BITS[depth]
        bit = 1 << t
        item_at_pos = np.empty(NET_N, np.int64)
        item_at_pos[cur] = items
        item_at_dst = np.empty(NET_N, np.int64)
        item_at_dst[dst] = items
        pin = item_at_pos[cur ^ bit]
        pout = item_at_dst[dst ^ bit]
        color = np.full(NET_N, -1, np.int8)
        for start in range(NET_N):
            if color[start] >= 0:
                continue
            i = start
            col = 0
            use_in = True
            while color[i] < 0:
                color[i] = col
                i = pin[i] if use_in else pout[i]
                use_in = not use_in
                col = 1 - col
        color = color.astype(np.int64)
        newc = (cur & ~bit) | (color << t)
        masks[depth][newc[newc != cur]] = 1
        up = N_STAGE - 1 - depth
        newd = (dst & ~bit) | (color << t)
        masks[up][dst[newd != dst]] = 1
        cur = newc
        dst = newd
    bit = 1 << ALL_BITS[HALF_D]
    diff = cur ^ dst
    assert np.all((diff & ~bit) == 0), "Benes middle-stage residual misrouting"
    masks[HALF_D][dst[diff != 0]] = 1
    return masks


def _build_assignment(j_batch):
    slots = np.arange(NSINK, dtype=np.int64)
    sink_addr = (slots >> 7) + 128 * (slots & 127)
    v = j_batch.astype(np.int64)
    counts = np.bincount(v, minlength=512)
    if counts.max() > 64:
        raise RuntimeError(f"idx multiplicity {counts.max()} > 64 unsupported")
    order = np.argsort(v, kind="stable")
    ranks = np.empty(NSINK, np.int64)
    start = np.concatenate([[0], np.cumsum(counts)[:-1]])
    ranks[order] = np.arange(NSINK) - np.repeat(start, counts)
    src_addr = (v & 127) + 128 * ranks + 8192 * (v >> 7)
    cur = np.empty(NET_N, np.int64)
    dst = np.empty(NET_N, np.int64)
    cur[:NSINK] = src_addr
    dst[:NSINK] = sink_addr
    used = np.zeros(NET_N, bool)
    used[src_addr] = True
    cur[NSINK:] = np.flatnonzero(~used)
    dst[NSINK:] = np.arange(NSINK, NET_N, dtype=np.int64)
    return cur, dst


def _stage_masks_2d(masks):
    """flat stage masks -> (fulls 15x[128,256], cpt [128,128], halfs 7x[128,128])."""
    a = np.arange(NET_N, dtype=np.int64)
    p_of = a & 127
    f_of = a >> 7
    fulls, halfs = [], []
    cpt = None
    for s in range(N_STAGE):
        if s in SKIP_STAGES:
            continue
        m = masks[s]
        if 8 <= s <= 20:  # G1: P' = f_low, F' = p + 128*f_high
            m2 = np.zeros((128, 256), np.uint8)
            m2[(a >> 7) & 127, (a & 127) + 128 * (a >> 14)] = m
            fulls.append(m2)
        elif s == 21:     # compacting up stage (bit 14): keep f < 128
            sel = f_of < 128
            m2 = np.zeros((128, 128), np.uint8)
            m2[p_of[sel], f_of[sel]] = m[sel]
            cpt = m2
        elif s >= 22:     # half stages
            sel = f_of < 128
            m2 = np.zeros((128, 128), np.uint8)
            m2[p_of[sel], f_of[sel]] = m[sel]
            halfs.append(m2)
        else:             # s = 6, 7 in G0
            m2 = np.zeros((128, 256), np.uint8)
            m2[p_of, f_of] = m
            fulls.append(m2)
    assert len(fulls) == 15 and len(halfs) == 7 and cpt is not None
    return fulls, cpt, halfs


def _pack_core_masks(mask_sets):
    """mask_sets: per-batch (fulls, cpt, halfs) -> mC [128, MB_TOT] uint8."""
    mC = np.zeros((128, MB_TOT), np.uint8)
    for si in range(15):
        for b in range(BPC):
            mC[:, _OFFS[si] + 256 * b : _OFFS[si] + 256 * (b + 1)] = mask_sets[b][0][si]
    for b in range(BPC):
        c = mask_sets[b][1]
        mC[:, _OFF_CPT_C + 128 * b : _OFF_CPT_C + 128 * (b + 1)] = c
        mC[:, _OFF_CPT_S + 128 * b : _OFF_CPT_S + 128 * (b + 1)] = 1 - c
    for si in range(7):
        for b in range(BPC):
            mC[:, _OFFS_H[si] + 128 * b : _OFFS_H[si] + 128 * (b + 1)] = (
                mask_sets[b][2][si]
            )
    return mC


# ---------------- device program ----------------

def _build_program():
    import concourse.bacc as bacc
    import concourse.mybir as mybir
    import concourse.tile as tile

    fp32 = mybir.dt.float32
    fp16 = mybir.dt.float16
    bf16 = mybir.dt.bfloat16
    u8 = mybir.dt.uint8

    nc = bacc.Bacc("TRN2", target_bir_lowering=False, debug=False)

    t2b = nc.dram_tensor("t2b", [BPC, 8, 128, 512], bf16, kind="ExternalInput")
    gP = nc.dram_tensor("gP", [128, 8], bf16, kind="ExternalInput")
    ident = nc.dram_tensor("ident", [128, 128], fp32, kind="ExternalInput")
    identb = nc.dram_tensor("identb", [128, 128], fp16, kind="ExternalInput")
    mCd = nc.dram_tensor("mC", [128, MB_TOT], u8, kind="ExternalInput")
    out = nc.dram_tensor("out", [BPC, 128, 128], fp32, kind="ExternalOutput")

    with tile.TileContext(nc) as tc, ExitStack() as ctx:
        constp = ctx.enter_context(tc.tile_pool(name="const", bufs=1))
        t2p = ctx.enter_context(tc.tile_pool(name="t2p", bufs=16))
        netp = ctx.enter_context(tc.tile_pool(name="net", bufs=1))
        smallp = ctx.enter_context(tc.tile_pool(name="small", bufs=2))
        psump = ctx.enter_context(tc.tile_pool(name="psum", bufs=2, space="PSUM"))
        psumu = ctx.enter_context(tc.tile_pool(name="psumu", bufs=1, space="PSUM"))

        # ---- DMA loads ----
        gP_t = constp.tile([128, 8], bf16)
        nc.sync.dma_start(gP_t[:], gP[:])
        ident_t = constp.tile([128, 128], fp32)
        nc.scalar.dma_start(ident_t[:], ident[:])
        identb_t = constp.tile([128, 128], fp16)
        nc.scalar.dma_start(identb_t[:], identb[:])
        mC_t = constp.tile([128, MB_TOT], u8)
        nc.gpsimd.dma_start(mC_t[:], mCd[:])

        # t2 chunk loads (host pre-transposed): [128 f, 512 n] per chunk
        t2T = {}
        for b in range(BPC):
            eng = nc.sync if b == 0 else nc.scalar
            for c in range(8):
                tt = t2p.tile([128, 512], bf16, tag="t2T", name=f"t2T_{b}_{c}")
                eng.dma_start(tt[:], t2b[b, c])
                t2T[(b, c)] = tt
        # masks enter the DMA pool only after t2 is issued (fair-share)
        mC_t = constp.tile([128, MB_TOT], u8)
        nc.sync.dma_start(mC_t[:], mCd[:])

        # ---- Benes stage helper: 2 in-place preds per stage ----
        X0 = netp.tile([128, 512], fp16, tag="net0", name="X0")
        T = netp.tile([128, 512], fp16, tag="tmp", name="Ttmp")
        nc.vector.memset(T[:], 0)

        def stage(Xap, width, fbit, moff, toff=0):
            lo = 1 << fbit
            Tap = T[:, toff : toff + width]
            X4 = Xap.rearrange("p (h b2 l) -> p h b2 l", b2=2, l=lo)
            T4 = Tap.rearrange("p (h b2 l) -> p h b2 l", b2=2, l=lo)
            M4 = mC_t[:, moff : moff + width].rearrange(
                "p (h b2 l) -> p h b2 l", b2=2, l=lo
            )
            # pass 1 (ACT): T = X  (bitcast fp32: halves element count)
            nc.scalar.copy(Tap.bitcast(fp32), Xap.bitcast(fp32))
            # pass 2 (DVE): X[pos] = T[pos^bit] where mC[pos]
            nc.vector.copy_predicated(X4, M4, T4[:, :, ::-1, :])

        # ---- u2 per batch: psum row -> 4 PE transposes -> exp(fp16) ----
        for b in range(BPC):
            u2row_ps = psumu.tile([1, 512], fp32, tag=f"u2r{b}")
            for c in range(8):
                nc.tensor.matmul(
                    u2row_ps[:],
                    gP_t[:, c : c + 1],
                    t2T[(b, c)][:],
                    start=(c == 0),
                    stop=(c == 7),
                )
            u2row = smallp.tile([1, 512], fp32, tag=f"u2row{b}")
            nc.scalar.copy(u2row[:], u2row_ps[:])
            pcols = psumu.tile([128, 4], fp32, tag=f"pcols{b}")
            for c in range(4):
                nc.tensor.matmul(
                    pcols[:, c : c + 1],
                    u2row[:, 128 * c : 128 * (c + 1)],
                    ident_t[0:1, 0:1],
                    is_transpose=True,
                )
            u2exp = smallp.tile([128, 4], fp16, tag=f"u2exp{b}")
            nc.scalar.activation(
                u2exp[:], pcols[:], func=mybir.ActivationFunctionType.Exp
            )
            # broadcast E over each 64-col block by log-doubling (stride-0
            # broadcast reads are ~10x slower on DVE)
            X4b = X0[:, 256 * b : 256 * (b + 1)].rearrange(
                "p (c r) -> p c r", r=64
            )
            nc.scalar.copy(X4b[:, :, 0:1], u2exp[:].unsqueeze(2))
            nc.scalar.copy(X4b[:, :, 1:2], X4b[:, :, 0:1])
            w = 2
            while w < 64:
                nc.scalar.copy(
                    X4b[:, :, w : 2 * w].bitcast(fp32),
                    X4b[:, :, 0:w].bitcast(fp32),
                )
                w *= 2
            for si in range(2):
                stage(
                    X0[:, 256 * b : 256 * (b + 1)], 256, FULL_FBITS[si],
                    _OFFS[si] + 256 * b, toff=256 * b,
                )

        # ---- Benes main chain ----
        cur = X0[:]
        for si in range(1, 15):
            if si >= 2:
                for b in range(BPC):
                    stage(
                        cur[:, 256 * b : 256 * (b + 1)], 256, FULL_FBITS[si],
                        _OFFS[si] + 256 * b, toff=256 * b,
                    )
            if si in CT_AFTER:
                Z = netp.tile([128, 512], fp16, tag=f"netct{si}", name=f"CT{si}")
                for blk in range(4):
                    pt = psump.tile([128, 128], fp16, tag="pt", name=f"pt_{si}_{blk}")
                    nc.tensor.transpose(
                        pt[:], cur[:, 128 * blk : 128 * (blk + 1)], identb_t[:]
                    )
                    if blk % 2 == 0:
                        nc.scalar.copy(Z[:, 128 * blk : 128 * (blk + 1)], pt[:])
                    else:
                        nc.vector.tensor_copy(
                            Z[:, 128 * blk : 128 * (blk + 1)], pt[:]
                        )
                cur = Z[:]

        # compacting up stage (flat bit 14): [128, 512] -> [128, 256]
        Yc = netp.tile([128, 256], fp16, tag="neth", name="Ycpt")
        for b2 in range(2):
            Yb = Yc[:, 128 * b2 : 128 * (b2 + 1)]
            Sb = mC_t[:, _OFF_CPT_S + 128 * b2 : _OFF_CPT_S + 128 * (b2 + 1)]
            nc.vector.copy_predicated(Yb, Sb, cur[:, 256 * b2 : 256 * b2 + 128])
        for b2 in range(2):
            Yb = Yc[:, 128 * b2 : 128 * (b2 + 1)]
            Cb = mC_t[:, _OFF_CPT_C + 128 * b2 : _OFF_CPT_C + 128 * (b2 + 1)]
            nc.vector.copy_predicated(Yb, Cb, cur[:, 256 * b2 + 128 : 256 * b2 + 256])
        cur = Yc[:]

        for si in range(7):
            for b in range(BPC):
                stage(
                    cur[:, 128 * b : 128 * (b + 1)], 128, HALF_FBITS[si],
                    _OFFS_H[si] + 128 * b, toff=128 * b,
                )

        # ---- stacked windowed softmax normalize + store ----
        C4v = cur.rearrange("p (Bv s d) -> p Bv s d", Bv=2, d=32)
        S = smallp.tile([128, 8], fp32, tag="S")
        nc.vector.tensor_reduce(
            out=S[:].rearrange("p (Bv s) -> p Bv s", Bv=2),
            in_=C4v,
            axis=mybir.AxisListType.X,
            op=mybir.AluOpType.add,
        )
        R = smallp.tile([128, 8], fp32, tag="R")
        nc.vector.reciprocal(R[:], S[:])
        Rf = smallp.tile([128, 256], fp16, tag="Rf")
        R3 = Rf[:].rearrange("p (w d) -> p w d", d=32)
        nc.vector.tensor_copy(R3[:, :, 0:1], R[:].unsqueeze(2))
        nc.vector.tensor_copy(R3[:, :, 1:2], R3[:, :, 0:1])
        w = 2
        while w < 32:
            nc.vector.tensor_copy(
                R3[:, :, w : 2 * w].bitcast(fp32), R3[:, :, 0:w].bitcast(fp32)
            )
            w *= 2
        O = smallp.tile([128, 256], fp32, tag="O")
        nc.vector.tensor_tensor(out=O[:], in0=cur, in1=Rf[:], op=mybir.AluOpType.mult)
        nc.sync.dma_start(out[0], O[:, 0:128])
        nc.scalar.dma_start(out[1], O[:, 128:256])

    nc.compile()
    return nc


# ---------------- host orchestration ----------------

def _compute_masks(idx_j):
    j3 = idx_j.reshape(B, N1 * DEG)
    per_batch = []
    for b in range(B):
        cur, dst = _build_assignment(j3[b])
        masks = _route_benes(cur, dst)
        per_batch.append(_stage_masks_2d(masks))
    return [
        _pack_core_masks(per_batch[BPC * c : BPC * (c + 1)]) for c in range(NCORES)
    ]


def _prep_core_inputs(t2, idx_j, W2, v):
    import ml_dtypes

    key = hashlib.sha256(np.ascontiguousarray(idx_j).tobytes()).hexdigest()
    if _CACHE.get("mask_key") != key:
        _CACHE["masks"] = _compute_masks(np.asarray(idx_j))
        _CACHE["mask_key"] = key
    mCs = _CACHE["masks"]

    g = (W2.T.astype(np.float64) @ v.astype(np.float64)).astype(np.float32)
    gPm = np.ascontiguousarray(g.reshape(8, 128).T).astype(ml_dtypes.bfloat16)
    ident = np.eye(128, dtype=np.float32)
    identb = np.eye(128, dtype=np.float16)

    in_maps = []
    for c in range(NCORES):
        bb = slice(BPC * c, BPC * (c + 1))
        in_maps.append(
            {
                "t2b": np.ascontiguousarray(
                    t2[bb].reshape(BPC, N2, 8, 128).transpose(0, 2, 3, 1)
                ).astype(ml_dtypes.bfloat16),
                "gP": gPm,
                "ident": ident,
                "identb": identb,
                "mC": mCs[c],
            }
        )
    return in_maps


def kernel(t1, t2, idx_b, idx_i, idx_j, W1, b1, W2, b2, v):
    from concourse.bass_utils import run_bass_kernel_spmd

    if "nc" not in _CACHE:
        _CACHE["nc"] = _build_program()
    nc = _CACHE["nc"]

    in_maps = _prep_core_inputs(
        np.asarray(t2, dtype=np.float32),
        np.asarray(idx_j),
        np.asarray(W2, dtype=np.float32),
        np.asarray(v, dtype=np.float32),
    )
    trace = bool(int(os.environ.get("KERNEL_TRACE", "0")))
    last_err = None
    for _attempt in range(3):
        try:
            res = run_bass_kernel_spmd(nc, in_maps, list(range(NCORES)), trace=trace)
            break
        except Exception as e:  # transient NRT_EXEC_UNIT_UNRECOVERABLE wedges
            last_err = e
    else:
        raise last_err
    _CACHE["last_results"] = res
    outs = [r["out"].reshape(BPC * N1 * DEG) for r in res.results]
    return np.concatenate(outs).astype(np.float32)


# revision 34
# speedup vs baseline: 1.0553x; 1.0553x over previous
"""Trainium2 Bass kernel for the sparse segment-softmax attention module.

Math: out[k] = segment_softmax((q1[b,i] + q2[b,j]) . v) over segments (b, i).
q1/b-bias terms cancel (softmax shift invariance), so
    out[k] = E[b, j_k] / sum_seg E,   E[b, n] = exp(t2[b, n, :] . g),
    g = W2^T v.  t1/W1/b1 are unused.

Device kernel per NeuronCore (2 of 16 batches, data-parallel over 8 cores):
  - t2 shard streams in bf16 TRANSPOSED (XBAR dma_start_transpose) so the
    PE computes u2 = t2 . g as 8 accumulating [128f x 512n] matmuls per
    batch into psum [1, 512]; 4 PE transposes turn the row into the
    [128, 4] table layout; exp on ACT -> bf16.
  - static Benes-network gather: the per-batch 16384-slot gather by idx_j
    is routed as a 15-bit Benes network (host-routed masks, uint8, one big
    DMA). Both batches stack in the free dim; each stage is TWO in-place
    DVE copy_predicated ops using reversed-stride pair views (bf16 data).
    Stages on addr bits 7..14 are free-dim selects; bits 0..6 run between
    two PE corner-turn transposes. Down stages on bits 7..12 pair
    identical values and are skipped. The bit-14 up stage compacts to the
    sink half; the last 7 stages run half width.
  - windowed softmax normalize (4 segments x 32 per partition) + store.

Output is produced directly in natural nnz order. Host does index routing
(cached by idx_j hash); no GPSIMD custom ops.
"""

import hashlib
import os
from contextlib import ExitStack

import numpy as np

B = 16
N1 = 512
N2 = 512
F2 = 1024
DEG = 32
NNZ = B * N1 * DEG
NCORES = 8
BPC = B // NCORES

# ---------------- Benes network topology (static) ----------------
NET_L = 15
NET_N = 1 << NET_L
NSINK = 16384
D_BITS = [7, 8, 9, 10, 11, 12, 13, 14, 0, 1, 2, 3, 4, 5]
M_BIT = 6
ALL_BITS = D_BITS + [M_BIT] + D_BITS[::-1]  # 29 stages
SKIP_STAGES = set(range(6))                 # identical-value pairs: no device op
N_STAGE = len(ALL_BITS)
HALF_D = (N_STAGE - 1) // 2

# Executed stages, device order:
#   idx 0..1   G0 full, f-bits [6, 7]            (flat bits 13, 14 down)
#   corner turn
#   idx 2..14  G1 full, f-bits [0,1,2,3,4,5,6,5,4,3,2,1,0]
#   corner turn
#   idx 15     G0 compacting up stage (flat bit 14): [128,512] -> [128,256]
#   idx 16..22 G0 half, f-bits [6,5,4,3,2,1,0]    (flat bits 13..7 up)
FULL_FBITS = [6, 7] + [0, 1, 2, 3, 4, 5, 6, 5, 4, 3, 2, 1, 0]  # 15 stages, w=512
HALF_FBITS = [6, 5, 4, 3, 2, 1, 0]                             # 7 stages, w=256
CT_AFTER = {1, 14}
_OFFS = [512 * i for i in range(15)]      # full-stage cross masks
_OFF_CPT_C = 15 * 512                     # compact stage cross mask [128, 256]
_OFF_CPT_S = _OFF_CPT_C + 256             # compact stage straight mask
_OFFS_H = [_OFF_CPT_S + 256 + 256 * i for i in range(7)]
MB_TOT = _OFFS_H[-1] + 256                # 9984

_CACHE: dict = {}


# ---------------- host-side Benes routing ----------------

def _route_benes(cur0, dst0):
    masks = [np.zeros(NET_N, np.uint8) for _ in range(N_STAGE)]
    cur = cur0.astype(np.int64).copy()
    dst = dst0.astype(np.int64).copy()
    items = np.arange(NET_N)
    for depth in range(HALF_D):
        t = ALL_BITS[depth]
        bit = 1 << t
        item_at_pos = np.empty(NET_N, np.int64)
        item_at_pos[cur] = items
        item_at_dst = np.empty(NET_N, np.int64)
        item_at_dst[dst] = items
        pin = item_at_pos[cur ^ bit]
        pout = item_at_dst[dst ^ bit]
        color = np.full(NET_N, -1, np.int8)
        for start in range(NET_N):
            if color[start] >= 0:
                continue
            i = start
            col = 0
            use_in = True
            while color[i] < 0:
                color[i] = col
                i = pin[i] if use_in else pout[i]
                use_in = not use_in
                col = 1 - col
        color = color.astype(np.int64)
        newc = (cur & ~bit) | (color << t)
        masks[depth][newc[newc != cur]] = 1
        up = N_STAGE - 1 - depth
        newd = (dst & ~bit) | (color << t)
        masks[up][dst[newd != dst]] = 1
        cur = newc
        dst = newd
    bit = 1 << ALL_BITS[HALF_D]
    diff = cur ^ dst
    assert np.all((diff & ~bit) == 0), "Benes middle-stage residual misrouting"
    masks[HALF_D][dst[diff != 0]] = 1
    return masks


def _build_assignment(j_batch):
    slots = np.arange(NSINK, dtype=np.int64)
    sink_addr = (slots >> 7) + 128 * (slots & 127)
    v = j_batch.astype(np.int64)
    counts = np.bincount(v, minlength=512)
    if counts.max() > 64:
        raise RuntimeError(f"idx multiplicity {counts.max()} > 64 unsupported")
    order = np.argsort(v, kind="stable")
    ranks = np.empty(NSINK, np.int64)
    start = np.concatenate([[0], np.cumsum(counts)[:-1]])
    ranks[order] = np.arange(NSINK) - np.repeat(start, counts)
    src_addr = (v & 127) + 128 * ranks + 8192 * (v >> 7)
    cur = np.empty(NET_N, np.int64)
    dst = np.empty(NET_N, np.int64)
    cur[:NSINK] = src_addr
    dst[:NSINK] = sink_addr
    used = np.zeros(NET_N, bool)
    used[src_addr] = True
    cur[NSINK:] = np.flatnonzero(~used)
    dst[NSINK:] = np.arange(NSINK, NET_N, dtype=np.int64)
    return cur, dst


def _stage_masks_2d(masks):
    """flat stage masks -> (fulls 15x[128,256], cpt [128,128], halfs 7x[128,128])."""
    a = np.arange(NET_N, dtype=np.int64)
    p_of = a & 127
    f_of = a >> 7
    fulls, halfs = [], []
    cpt = None
    for s in range(N_STAGE):
        if s in SKIP_STAGES:
            continue
        m = masks[s]
        if 8 <= s <= 20:  # G1: P' = f_low, F' = p + 128*f_high
            m2 = np.zeros((128, 256), np.uint8)
            m2[(a >> 7) & 127, (a & 127) + 128 * (a >> 14)] = m
            fulls.append(m2)
        elif s == 21:     # compacting up stage (bit 14): keep f < 128
            sel = f_of < 128
            m2 = np.zeros((128, 128), np.uint8)
            m2[p_of[sel], f_of[sel]] = m[sel]
            cpt = m2
        elif s >= 22:     # half stages
            sel = f_of < 128
            m2 = np.zeros((128, 128), np.uint8)
            m2[p_of[sel], f_of[sel]] = m[sel]
            halfs.append(m2)
        else:             # s = 6, 7 in G0
            m2 = np.zeros((128, 256), np.uint8)
            m2[p_of, f_of] = m
            fulls.append(m2)
    assert len(fulls) == 15 and len(halfs) == 7 and cpt is not None
    return fulls, cpt, halfs


def _pack_core_masks(mask_sets):
    """mask_sets: per-batch (fulls, cpt, halfs) -> mC [128, MB_TOT] uint8."""
    mC = np.zeros((128, MB_TOT), np.uint8)
    for si in range(15):
        for b in range(BPC):
            mC[:, _OFFS[si] + 256 * b : _OFFS[si] + 256 * (b + 1)] = mask_sets[b][0][si]
    for b in range(BPC):
        c = mask_sets[b][1]
        mC[:, _OFF_CPT_C + 128 * b : _OFF_CPT_C + 128 * (b + 1)] = c
        mC[:, _OFF_CPT_S + 128 * b : _OFF_CPT_S + 128 * (b + 1)] = 1 - c
    for si in range(7):
        for b in range(BPC):
            mC[:, _OFFS_H[si] + 128 * b : _OFFS_H[si] + 128 * (b + 1)] = (
                mask_sets[b][2][si]
            )
    return mC


# ---------------- device program ----------------

def _build_program():
    import concourse.bacc as bacc
    import concourse.mybir as mybir
    import concourse.tile as tile

    fp32 = mybir.dt.float32
    fp16 = mybir.dt.float16
    bf16 = mybir.dt.bfloat16
    u8 = mybir.dt.uint8

    nc = bacc.Bacc("TRN2", target_bir_lowering=False, debug=False)

    t2b = nc.dram_tensor("t2b", [BPC, 8, 128, 512], bf16, kind="ExternalInput")
    gP = nc.dram_tensor("gP", [128, 8], bf16, kind="ExternalInput")
    ident = nc.dram_tensor("ident", [128, 128], fp32, kind="ExternalInput")
    identb = nc.dram_tensor("identb", [128, 128], fp16, kind="ExternalInput")
    mCd = nc.dram_tensor("mC", [128, MB_TOT], u8, kind="ExternalInput")
    out = nc.dram_tensor("out", [BPC, 128, 128], fp32, kind="ExternalOutput")

    with tile.TileContext(nc) as tc, ExitStack() as ctx:
        constp = ctx.enter_context(tc.tile_pool(name="const", bufs=1))
        t2p = ctx.enter_context(tc.tile_pool(name="t2p", bufs=16))
        netp = ctx.enter_context(tc.tile_pool(name="net", bufs=1))
        smallp = ctx.enter_context(tc.tile_pool(name="small", bufs=2))
        psump = ctx.enter_context(tc.tile_pool(name="psum", bufs=2, space="PSUM"))
        psumu = ctx.enter_context(tc.tile_pool(name="psumu", bufs=1, space="PSUM"))

        # ---- DMA loads ----
        gP_t = constp.tile([128, 8], bf16)
        nc.sync.dma_start(gP_t[:], gP[:])
        ident_t = constp.tile([128, 128], fp32)
        nc.scalar.dma_start(ident_t[:], ident[:])
        identb_t = constp.tile([128, 128], fp16)
        nc.scalar.dma_start(identb_t[:], identb[:])
        mC_t = constp.tile([128, MB_TOT], u8)
        nc.gpsimd.dma_start(mC_t[:], mCd[:])

        mC_t = constp.tile([128, MB_TOT], u8)
        nc.gpsimd.dma_start(mC_t[:], mCd[:])
        # t2 chunk loads (host pre-transposed): [128 f, 512 n] per chunk
        t2T = {}
        for b in range(BPC):
            eng = nc.sync if b == 0 else nc.scalar
            for c in range(8):
                tt = t2p.tile([128, 512], bf16, tag="t2T", name=f"t2T_{b}_{c}")
                eng.dma_start(tt[:], t2b[b, c])
                t2T[(b, c)] = tt

        # ---- Benes stage helper: 2 in-place preds per stage ----
        X0 = netp.tile([128, 512], fp16, tag="net0", name="X0")
        T = netp.tile([128, 512], fp16, tag="tmp", name="Ttmp")
        nc.vector.memset(T[:], 0)

        def stage(Xap, width, fbit, moff, toff=0):
            lo = 1 << fbit
            Tap = T[:, toff : toff + width]
            X4 = Xap.rearrange("p (h b2 l) -> p h b2 l", b2=2, l=lo)
            T4 = Tap.rearrange("p (h b2 l) -> p h b2 l", b2=2, l=lo)
            M4 = mC_t[:, moff : moff + width].rearrange(
                "p (h b2 l) -> p h b2 l", b2=2, l=lo
            )
            # pass 1 (ACT): T = X  (bitcast fp32: halves element count)
            nc.scalar.copy(Tap.bitcast(fp32), Xap.bitcast(fp32))
            # pass 2 (DVE): X[pos] = T[pos^bit] where mC[pos]
            nc.vector.copy_predicated(X4, M4, T4[:, :, ::-1, :])

        # ---- u2 per batch: psum row -> 4 PE transposes -> exp(fp16) ----
        for b in range(BPC):
            u2row_ps = psumu.tile([1, 512], fp32, tag=f"u2r{b}")
            for c in range(8):
                nc.tensor.matmul(
                    u2row_ps[:],
                    gP_t[:, c : c + 1],
                    t2T[(b, c)][:],
                    start=(c == 0),
                    stop=(c == 7),
                )
            u2row = smallp.tile([1, 512], fp32, tag=f"u2row{b}")
            nc.scalar.copy(u2row[:], u2row_ps[:])
            pcols = psumu.tile([128, 4], fp32, tag=f"pcols{b}")
            for c in range(4):
                nc.tensor.matmul(
                    pcols[:, c : c + 1],
                    u2row[:, 128 * c : 128 * (c + 1)],
                    ident_t[0:1, 0:1],
                    is_transpose=True,
                )
            u2exp = smallp.tile([128, 4], fp16, tag=f"u2exp{b}")
            nc.scalar.activation(
                u2exp[:], pcols[:], func=mybir.ActivationFunctionType.Exp
            )
            # broadcast E over each 64-col block by log-doubling (stride-0
            # broadcast reads are ~10x slower on DVE)
            X4b = X0[:, 256 * b : 256 * (b + 1)].rearrange(
                "p (c r) -> p c r", r=64
            )
            nc.vector.tensor_copy(X4b[:, :, 0:1], u2exp[:].unsqueeze(2))
            nc.vector.tensor_copy(X4b[:, :, 1:2], X4b[:, :, 0:1])
            w = 2
            while w < 64:
                nc.vector.tensor_copy(
                    X4b[:, :, w : 2 * w].bitcast(fp32),
                    X4b[:, :, 0:w].bitcast(fp32),
                )
                w *= 2
            for si in range(2):
                stage(
                    X0[:, 256 * b : 256 * (b + 1)], 256, FULL_FBITS[si],
                    _OFFS[si] + 256 * b, toff=256 * b,
                )

        # ---- Benes main chain ----
        cur = X0[:]
        for si in range(1, 15):
            if si >= 2:
                for b in range(BPC):
                    stage(
                        cur[:, 256 * b : 256 * (b + 1)], 256, FULL_FBITS[si],
                        _OFFS[si] + 256 * b, toff=256 * b,
                    )
            if si in CT_AFTER:
                Z = netp.tile([128, 512], fp16, tag=f"netct{si}", name=f"CT{si}")
                for blk in range(4):
                    pt = psump.tile([128, 128], fp16, tag="pt", name=f"pt_{si}_{blk}")
                    nc.tensor.transpose(
                        pt[:], cur[:, 128 * blk : 128 * (blk + 1)], identb_t[:]
                    )
                    if blk % 2 == 0:
                        nc.scalar.copy(Z[:, 128 * blk : 128 * (blk + 1)], pt[:])
                    else:
                        nc.vector.tensor_copy(
                            Z[:, 128 * blk : 128 * (blk + 1)], pt[:]
                        )
                cur = Z[:]

        # compacting up stage (flat bit 14): [128, 512] -> [128, 256]
        Yc = netp.tile([128, 256], fp16, tag="neth", name="Ycpt")
        for b2 in range(2):
            Yb = Yc[:, 128 * b2 : 128 * (b2 + 1)]
            Sb = mC_t[:, _OFF_CPT_S + 128 * b2 : _OFF_CPT_S + 128 * (b2 + 1)]
            nc.vector.copy_predicated(Yb, Sb, cur[:, 256 * b2 : 256 * b2 + 128])
        for b2 in range(2):
            Yb = Yc[:, 128 * b2 : 128 * (b2 + 1)]
            Cb = mC_t[:, _OFF_CPT_C + 128 * b2 : _OFF_CPT_C + 128 * (b2 + 1)]
            nc.vector.copy_predicated(Yb, Cb, cur[:, 256 * b2 + 128 : 256 * b2 + 256])
        cur = Yc[:]

        for si in range(7):
            for b in range(BPC):
                stage(
                    cur[:, 128 * b : 128 * (b + 1)], 128, HALF_FBITS[si],
                    _OFFS_H[si] + 128 * b, toff=128 * b,
                )

        # ---- stacked windowed softmax normalize + store ----
        C4v = cur.rearrange("p (Bv s d) -> p Bv s d", Bv=2, d=32)
        S = smallp.tile([128, 8], fp32, tag="S")
        nc.vector.tensor_reduce(
            out=S[:].rearrange("p (Bv s) -> p Bv s", Bv=2),
            in_=C4v,
            axis=mybir.AxisListType.X,
            op=mybir.AluOpType.add,
        )
        R = smallp.tile([128, 8], fp32, tag="R")
        nc.vector.reciprocal(R[:], S[:])
        Rf = smallp.tile([128, 256], fp16, tag="Rf")
        R3 = Rf[:].rearrange("p (w d) -> p w d", d=32)
        nc.vector.tensor_copy(R3[:, :, 0:1], R[:].unsqueeze(2))
        nc.vector.tensor_copy(R3[:, :, 1:2], R3[:, :, 0:1])
        w = 2
        while w < 32:
            nc.vector.tensor_copy(
                R3[:, :, w : 2 * w].bitcast(fp32), R3[:, :, 0:w].bitcast(fp32)
            )
            w *= 2
        O = smallp.tile([128, 256], fp32, tag="O")
        nc.vector.tensor_tensor(out=O[:], in0=cur, in1=Rf[:], op=mybir.AluOpType.mult)
        nc.sync.dma_start(out[0], O[:, 0:128])
        nc.scalar.dma_start(out[1], O[:, 128:256])

    nc.compile()
    return nc


# ---------------- host orchestration ----------------

def _compute_masks(idx_j):
    j3 = idx_j.reshape(B, N1 * DEG)
    per_batch = []
    for b in range(B):
        cur, dst = _build_assignment(j3[b])
        masks = _route_benes(cur, dst)
        per_batch.append(_stage_masks_2d(masks))
    return [
        _pack_core_masks(per_batch[BPC * c : BPC * (c + 1)]) for c in range(NCORES)
    ]


def _prep_core_inputs(t2, idx_j, W2, v):
    import ml_dtypes

    key = hashlib.sha256(np.ascontiguousarray(idx_j).tobytes()).hexdigest()
    if _CACHE.get("mask_key") != key:
        _CACHE["masks"] = _compute_masks(np.asarray(idx_j))
        _CACHE["mask_key"] = key
    mCs = _CACHE["masks"]

    g = (W2.T.astype(np.float64) @ v.astype(np.float64)).astype(np.float32)
    gPm = np.ascontiguousarray(g.reshape(8, 128).T).astype(ml_dtypes.bfloat16)
    ident = np.eye(128, dtype=np.float32)
    identb = np.eye(128, dtype=np.float16)

    in_maps = []
    for c in range(NCORES):
        bb = slice(BPC * c, BPC * (c + 1))
        in_maps.append(
            {
                "t2b": np.ascontiguousarray(
                    t2[bb].reshape(BPC, N2, 8, 128).transpose(0, 2, 3, 1)
                ).astype(ml_dtypes.bfloat16),
                "gP": gPm,
                "ident": ident,
                "identb": identb,
                "mC": mCs[c],
            }
        )
    return in_maps


def kernel(t1, t2, idx_b, idx_i, idx_j, W1, b1, W2, b2, v):
    from concourse.bass_utils import run_bass_kernel_spmd

    if "nc" not in _CACHE:
        _CACHE["nc"] = _build_program()
    nc = _CACHE["nc"]

    in_maps = _prep_core_inputs(
        np.asarray(t2, dtype=np.float32),
        np.asarray(idx_j),
        np.asarray(W2, dtype=np.float32),
        np.asarray(v, dtype=np.float32),
    )
    trace = bool(int(os.environ.get("KERNEL_TRACE", "0")))
    last_err = None
    for _attempt in range(3):
        try:
            res = run_bass_kernel_spmd(nc, in_maps, list(range(NCORES)), trace=trace)
            break
        except Exception as e:  # transient NRT_EXEC_UNIT_UNRECOVERABLE wedges
            last_err = e
    else:
        raise last_err
    _CACHE["last_results"] = res
    outs = [r["out"].reshape(BPC * N1 * DEG) for r in res.results]
    return np.concatenate(outs).astype(np.float32)
